# revision 6
# baseline (speedup 1.0000x reference)
"""Trainium2 Bass kernel for nn_FepNet_46342697123845.

Strategy: data-parallel over batch B (64 -> 8 rows per NeuronCore), weights
replicated. The recurrence is numerically chaotic (a single Bernoulli sample
flip cascades), so every fp32 operation replicates the XLA-neuron lowering of
the reference bit-for-bit:
  - dots: out^T orientation [128h x B], W-side stationary (fp32 LOW/HIGH
    decomposition on the weight), K-chunks 0..7 accumulated sequentially in
    PSUM (verified bitwise vs jit(x @ W) on device);
  - elementwise: ACT-engine affine path for scale/bias folds
    (0.9*p, 0.1*pre, exp(-x), +1.0, 0.99*x+0.005), DVE for add/recip/cmp
    (verified bitwise vs the jitted chain);
  - RNG: the neuron backend lowers jax.random to a hardware DVE RNG; the
    per-step uniforms are harvested on the host via a jitted scan of
    jax.random.uniform over the same keys (bit-identical lowering).

Phases: A) XW^T = (x_t @ W_ih)^T for all t (parallel over t), through a DRAM
scratch; B) sequential scan with spike^T state kept H-major on chip;
C) preds = sigmoid(probs @ W_hh^T) batched over all t.
"""

import os
import numpy as np

T, B, I, H = 512, 64, 1024, 1024
NC = 8                      # cores
BL = B // NC                # batch rows per core = 8
KT = 8                      # K chunks (1024/128)
HC = 8                      # H chunks
TG = 16                     # t-groups
TB = T // TG                # steps per group = 32
NG = int(os.environ.get("KERNEL_NG", "2"))  # h-chunk groups per scan step
GW = HC // NG

_BUILD_CACHE = {}


def _build():
    if "nc" in _BUILD_CACHE:
        return _BUILD_CACHE["nc"]
    import concourse.bass as bass
    import concourse.tile as tile
    from concourse import bacc, mybir

    f32 = mybir.dt.float32
    ACT = mybir.ActivationFunctionType
    ALU = mybir.AluOpType
    S01 = 0.10000000149011612    # fp32(0.1)
    S099 = 0.9900000095367432    # fp32(0.99)
    S0005 = 0.004999999888241291  # fp32(0.005)

    nc = bacc.Bacc("TRN2", target_bir_lowering=False, debug=False, num_devices=1)

    xT = nc.dram_tensor("xT", (I, T, BL), f32, kind="ExternalInput").ap()
    ublk = nc.dram_tensor("ublk", (TG, HC, 128, TB, BL), f32, kind="ExternalInput").ap()
    w_ih = nc.dram_tensor("w_ih", (I, H), f32, kind="ExternalInput").ap()
    w_hh = nc.dram_tensor("w_hh", (H, H), f32, kind="ExternalInput").ap()
    w_hht = nc.dram_tensor("w_hht", (H, H), f32, kind="ExternalInput").ap()

    probs_blk = nc.dram_tensor("probs_blk", (TG, HC, 128, TB, BL), f32, kind="ExternalOutput").ap()
    spikes_blk = nc.dram_tensor("spikes_blk", (TG, HC, 128, TB, BL), f32, kind="ExternalOutput").ap()
    preds_blk = nc.dram_tensor("preds_blk", (TG, HC, 128, TB, BL), f32, kind="ExternalOutput").ap()

    with tile.TileContext(nc) as tc:
        from contextlib import ExitStack
        with ExitStack() as ctx:
            dram = ctx.enter_context(tc.tile_pool(name="dram", bufs=1, space="DRAM"))
            wpool = ctx.enter_context(tc.tile_pool(name="w", bufs=2))
            xkp = ctx.enter_context(tc.tile_pool(name="xk", bufs=2))
            psA = ctx.enter_context(tc.tile_pool(name="psA", bufs=2, space="PSUM"))
            cpA = ctx.enter_context(tc.tile_pool(name="cpA", bufs=3))
            blkin = ctx.enter_context(tc.tile_pool(name="blkin", bufs=2))
            stg = ctx.enter_context(tc.tile_pool(name="stg", bufs=2))
            psB = ctx.enter_context(tc.tile_pool(name="psB", bufs=2, space="PSUM"))
            ch = ctx.enter_context(tc.tile_pool(name="ch", bufs=3))
            init = ctx.enter_context(tc.tile_pool(name="init", bufs=1))

            # scratch for XW^T blocks and a tracked copy of probs for phase C
            xw_scr = dram.tile([TG, HC, 128, TB, BL], f32)
            probs_scr = dram.tile([TG, HC, 128, TB, BL], f32)

            wih_sb = wpool.tile([128, KT, H], f32, tag="w")
            whh_sb = wpool.tile([128, KT, H], f32, tag="w")
            nc.sync.dma_start(out=wih_sb, in_=w_ih.rearrange("(k p) h -> p k h", p=128))
            nc.sync.dma_start(out=whh_sb, in_=w_hh.rearrange("(k p) h -> p k h", p=128))

            # ---------------- Phase A: XW^T ----------------
            xT_r = xT.rearrange("(k p) t b -> p k t b", p=128)
            for tg in range(TG):
                xk = xkp.tile([128, KT, TB, BL], f32)
                nc.sync.dma_start(out=xk, in_=xT_r[:, :, tg * TB:(tg + 1) * TB, :])
                for hc in range(HC):
                    ps = psA.tile([128, TB, BL], f32)
                    for k in range(KT):
                        nc.tensor.matmul(
                            ps,
                            wih_sb[:, k, hc * 128:(hc + 1) * 128],
                            xk[:, k, :, :],
                            start=(k == 0), stop=(k == KT - 1),
                        )
                    cp = cpA.tile([128, TB, BL], f32)
                    nc.vector.tensor_copy(cp, ps)
                    nc.sync.dma_start(out=xw_scr[tg, hc], in_=cp)

            # ---------------- Phase B: sequential scan ----------------
            probT = init.tile([128, HC, BL], f32)
            spikeT = init.tile([128, HC, BL], f32)
            nc.vector.memset(probT, 0.0)
            nc.vector.memset(spikeT, 0.0)
            prob_state = probT
            spike_state = spikeT

            for tg in range(TG):
                xwb = blkin.tile([128, HC, TB, BL], f32, tag="xwb")
                nc.sync.dma_start(out=xwb, in_=xw_scr[tg].rearrange("h p t b -> p h t b"))
                ub = blkin.tile([128, HC, TB, BL], f32, tag="ub")
                nc.sync.dma_start(out=ub, in_=ublk[tg].rearrange("h p t b -> p h t b"))
                pstg = stg.tile([128, HC, TB, BL], f32, tag="pstg")
                sstg = stg.tile([128, HC, TB, BL], f32, tag="sstg")

                for ti in range(TB):
                    # two h-chunk groups on separate PSUM banks: group 0's
                    # elementwise chain (and its spike chunks) completes while
                    # PE still runs group 1's weight loads, so the next step's
                    # k=0..3 matmuls start without waiting for the full chain
                    pss = [psB.tile([128, GW, BL], f32, tag=f"ps{g}", name=f"ps{g}_{tg}_{ti}")
                           for g in range(NG)]
                    for g in range(NG):
                        for hg in range(GW):
                            hc = g * GW + hg
                            for k in range(KT):
                                nc.tensor.matmul(
                                    pss[g][:, hg, :],
                                    whh_sb[:, k, hc * 128:(hc + 1) * 128],
                                    spike_state[:, k, :],
                                    start=(k == 0), stop=(k == KT - 1),
                                )
                    prob_new = pstg[:, :, ti, :]
                    spike_new = sstg[:, :, ti, :]
                    for g in range(NG):
                        hs = slice(g * GW, (g + 1) * GW)
                        pre = ch.tile([128, GW, BL], f32, tag=f"pre{g}")
                        nc.vector.tensor_add(pre, xwb[:, hs, ti, :], pss[g])
                        ta = ch.tile([128, GW, BL], f32, tag=f"ta{g}")
                        nc.scalar.activation(out=ta, in_=prob_state[:, hs, :], func=ACT.Copy, scale=0.9)
                        tb_ = ch.tile([128, GW, BL], f32, tag=f"tb{g}")
                        nc.scalar.activation(out=tb_, in_=pre, func=ACT.Copy, scale=S01)
                        arg = ch.tile([128, GW, BL], f32, tag=f"arg{g}")
                        nc.vector.tensor_add(arg, ta, tb_)
                        e = ch.tile([128, GW, BL], f32, tag=f"e{g}")
                        nc.scalar.activation(out=e, in_=arg, func=ACT.Exp, scale=-1.0)
                        e2 = ch.tile([128, GW, BL], f32, tag=f"e2{g}")
                        nc.scalar.activation(out=e2, in_=e, func=ACT.Copy, scale=1.0, bias=1.0)
                        sig = ch.tile([128, GW, BL], f32, tag=f"sig{g}")
                        nc.vector.reciprocal(sig, e2)
                        nc.scalar.activation(out=prob_new[:, hs, :], in_=sig, func=ACT.Copy, scale=S099, bias=S0005)
                        samp = ch.tile([128, GW, BL], f32, tag=f"samp{g}")
                        nc.vector.tensor_tensor(samp, ub[:, hs, ti, :], prob_new[:, hs, :], ALU.is_lt)
                        s1 = ch.tile([128, GW, BL], f32, tag=f"s1{g}")
                        nc.vector.tensor_add(s1, samp, prob_new[:, hs, :])
                        nc.vector.tensor_sub(spike_new[:, hs, :], s1, prob_new[:, hs, :])
                    prob_state = prob_new
                    spike_state = spike_new

                nc.sync.dma_start(out=probs_blk[tg].rearrange("h p t b -> p h t b"), in_=pstg)
                nc.sync.dma_start(out=probs_scr[tg].rearrange("h p t b -> p h t b"), in_=pstg)
                nc.sync.dma_start(out=spikes_blk[tg].rearrange("h p t b -> p h t b"), in_=sstg)

            # ---------------- Phase C: preds ----------------
            whht_sb = wpool.tile([128, KT, H], f32, tag="w")
            nc.sync.dma_start(out=whht_sb, in_=w_hht.rearrange("(k p) h -> p k h", p=128))
            for tg in range(TG):
                pk = xkp.tile([128, KT, TB, BL], f32)
                nc.sync.dma_start(out=pk, in_=probs_scr[tg].rearrange("k p t b -> p k t b"))
                for hc in range(HC):
                    ps = psA.tile([128, TB, BL], f32)
                    for k in range(KT):
                        nc.tensor.matmul(
                            ps,
                            whht_sb[:, k, hc * 128:(hc + 1) * 128],
                            pk[:, k, :, :],
                            start=(k == 0), stop=(k == KT - 1),
                        )
                    e = cpA.tile([128, TB, BL], f32, tag="eC")
                    nc.scalar.activation(out=e, in_=ps, func=ACT.Exp, scale=-1.0)
                    e2 = cpA.tile([128, TB, BL], f32, tag="e2C")
                    nc.scalar.activation(out=e2, in_=e, func=ACT.Copy, scale=1.0, bias=1.0)
                    pr = cpA.tile([128, TB, BL], f32, tag="prC")
                    nc.vector.reciprocal(pr, e2)
                    nc.sync.dma_start(out=preds_blk[tg, hc], in_=pr)

    nc.compile()
    _BUILD_CACHE["nc"] = nc
    return nc


def _harvest_uniforms():
    """Device-RNG uniforms, bit-identical to the scan's rng_bit_generator."""
    cache_path = os.path.join(os.path.dirname(os.path.abspath(__file__)), "cache", "U_dev.npy")
    if os.path.exists(cache_path):
        return np.load(cache_path)
    import jax, jax.numpy as jnp
    keys = jax.random.split(jax.random.key(42), T)

    def uscan(keys):
        def body(c, k):
            return c, jax.random.uniform(k, (B, H), dtype=jnp.float32)
        _, us = jax.lax.scan(body, 0, keys)
        return us
    return np.asarray(jax.jit(uscan)(keys))


def _blk(a):
    """[T, H, BL] -> [TG, HC, 128, TB, BL] block layout."""
    return np.ascontiguousarray(
        a.reshape(TG, TB, HC, 128, BL).transpose(0, 2, 3, 1, 4))


def _unblk(a):
    """[TG, HC, 128, TB, BL] -> [T, BL, H]."""
    return a.transpose(0, 3, 4, 1, 2).reshape(T, BL, H)


def kernel(inputs, W_ih, W_hh):
    from concourse.bass_utils import run_bass_kernel_spmd

    inputs = np.asarray(inputs, dtype=np.float32)
    W_ih = np.ascontiguousarray(np.asarray(W_ih, dtype=np.float32))
    W_hh = np.ascontiguousarray(np.asarray(W_hh, dtype=np.float32))
    W_hhT = np.ascontiguousarray(W_hh.T)

    U = _harvest_uniforms()           # [T, B, H]
    nc = _build()

    in_maps = []
    for c in range(NC):
        bsl = slice(c * BL, (c + 1) * BL)
        xTc = np.ascontiguousarray(inputs[:, bsl, :].transpose(2, 0, 1))   # [I, T, BL]
        u_thb = U[:, bsl, :].transpose(0, 2, 1)                            # [T, H, BL]
        ublk = _blk(u_thb)
        in_maps.append({
            "xT": xTc, "ublk": ublk,
            "w_ih": W_ih, "w_hh": W_hh, "w_hht": W_hhT,
        })

    _BUILD_CACHE["last_in_maps"] = in_maps
    res = run_bass_kernel_spmd(nc, in_maps, core_ids=list(range(NC)))

    probs = np.empty((T, B, H), np.float32)
    spikes = np.empty((T + 1, B, H), np.float32)
    preds = np.empty((T, B, H), np.float32)
    spikes[0] = 0.0
    for c in range(NC):
        bsl = slice(c * BL, (c + 1) * BL)
        r = res.results[c]
        probs[:, bsl, :] = _unblk(r["probs_blk"])
        spikes[1:, bsl, :] = _unblk(r["spikes_blk"])
        preds[:, bsl, :] = _unblk(r["preds_blk"])
    prob_T = probs[-1].copy()
    spike_T = spikes[-1].copy()
    return spikes, preds, probs, prob_T, spike_T


# revision 9
# speedup vs baseline: 3.6030x; 3.6030x over previous
"""Trainium2 Bass kernel for nn_FepNet_46342697123845.

Strategy: data-parallel over batch B (64 -> 8 rows per NeuronCore), weights
replicated. The recurrence is numerically chaotic (a single Bernoulli sample
flip cascades), so every fp32 operation replicates the XLA-neuron lowering of
the reference bit-for-bit:
  - dots: out^T orientation [128h x B], W-side stationary (fp32 LOW/HIGH
    decomposition on the weight), K-chunks 0..7 accumulated sequentially in
    PSUM (verified bitwise vs jit(x @ W) on device);
  - elementwise: ACT-engine affine path for scale/bias folds
    (0.9*p, 0.1*pre, exp(-x), +1.0, 0.99*x+0.005), DVE for add/recip/cmp
    (verified bitwise vs the jitted chain);
  - RNG: the neuron backend lowers jax.random to a hardware DVE RNG; the
    per-step uniforms are harvested on the host via a jitted scan of
    jax.random.uniform over the same keys (bit-identical lowering).

Phases: A) XW^T = (x_t @ W_ih)^T for all t (parallel over t), through a DRAM
scratch; B) sequential scan with spike^T state kept H-major on chip;
C) preds = sigmoid(probs @ W_hh^T) batched over all t.
"""

import os
import numpy as np

T, B, I, H = 512, 64, 1024, 1024
NC = 8                      # cores
BL = B // NC                # batch rows per core = 8
KT = 8                      # K chunks (1024/128)
HC = 8                      # H chunks
TG = 16                     # t-groups
TB = T // TG                # steps per group = 32
NG = int(os.environ.get("KERNEL_NG", "2"))  # h-chunk groups per scan step
GW = HC // NG

_BUILD_CACHE = {}


def _build():
    if "nc" in _BUILD_CACHE:
        return _BUILD_CACHE["nc"]
    import concourse.bass as bass
    import concourse.tile as tile
    from concourse import bacc, mybir

    f32 = mybir.dt.float32
    ACT = mybir.ActivationFunctionType
    ALU = mybir.AluOpType
    S01 = 0.10000000149011612    # fp32(0.1)
    S099 = 0.9900000095367432    # fp32(0.99)
    S0005 = 0.004999999888241291  # fp32(0.005)

    nc = bacc.Bacc("TRN2", target_bir_lowering=False, debug=False, num_devices=1)

    xT = nc.dram_tensor("xT", (I, T, BL), f32, kind="ExternalInput").ap()
    ublk = nc.dram_tensor("ublk", (TG, HC, 128, TB, BL), f32, kind="ExternalInput").ap()
    w_ih = nc.dram_tensor("w_ih", (I, H), f32, kind="ExternalInput").ap()
    w_hh = nc.dram_tensor("w_hh", (H, H), f32, kind="ExternalInput").ap()
    w_hht = nc.dram_tensor("w_hht", (H, H), f32, kind="ExternalInput").ap()

    probs_blk = nc.dram_tensor("probs_blk", (TG, HC, 128, TB, BL), f32, kind="ExternalOutput").ap()
    spikes_blk = nc.dram_tensor("spikes_blk", (TG, HC, 128, TB, BL), f32, kind="ExternalOutput").ap()
    preds_blk = nc.dram_tensor("preds_blk", (TG, HC, 128, TB, BL), f32, kind="ExternalOutput").ap()

    with tile.TileContext(nc) as tc:
        from contextlib import ExitStack
        with ExitStack() as ctx:
            dram = ctx.enter_context(tc.tile_pool(name="dram", bufs=1, space="DRAM"))
            wpool = ctx.enter_context(tc.tile_pool(name="w", bufs=2))
            xkp = ctx.enter_context(tc.tile_pool(name="xk", bufs=2))
            psA = ctx.enter_context(tc.tile_pool(name="psA", bufs=2, space="PSUM"))
            cpA = ctx.enter_context(tc.tile_pool(name="cpA", bufs=3))
            blkin = ctx.enter_context(tc.tile_pool(name="blkin", bufs=2))
            stg = ctx.enter_context(tc.tile_pool(name="stg", bufs=2))
            psB = ctx.enter_context(tc.tile_pool(name="psB", bufs=int(os.environ.get("KERNEL_PSB", "2")), space="PSUM"))
            ch = ctx.enter_context(tc.tile_pool(name="ch", bufs=3))
            init = ctx.enter_context(tc.tile_pool(name="init", bufs=1))

            # scratch for XW^T blocks and a tracked copy of probs for phase C
            xw_scr = dram.tile([TG, HC, 128, TB, BL], f32)
            probs_scr = dram.tile([TG, HC, 128, TB, BL], f32)

            wih_sb = wpool.tile([128, KT, H], f32, tag="w")
            whh_sb = wpool.tile([128, KT, H], f32, tag="w")
            nc.sync.dma_start(out=wih_sb, in_=w_ih.rearrange("(k p) h -> p k h", p=128))
            nc.sync.dma_start(out=whh_sb, in_=w_hh.rearrange("(k p) h -> p k h", p=128))

            # ---------------- Phase A: XW^T ----------------
            xT_r = xT.rearrange("(k p) t b -> p k t b", p=128)
            for tg in range(TG):
                xk = xkp.tile([128, KT, TB, BL], f32)
                nc.sync.dma_start(out=xk, in_=xT_r[:, :, tg * TB:(tg + 1) * TB, :])
                for hc in range(HC):
                    ps = psA.tile([128, TB, BL], f32)
                    for k in range(KT):
                        nc.tensor.matmul(
                            ps,
                            wih_sb[:, k, hc * 128:(hc + 1) * 128],
                            xk[:, k, :, :],
                            start=(k == 0), stop=(k == KT - 1),
                        )
                    cp = cpA.tile([128, TB, BL], f32)
                    nc.vector.tensor_copy(cp, ps)
                    nc.sync.dma_start(out=xw_scr[tg, hc], in_=cp)

            # ---------------- Phase B: sequential scan ----------------
            probT = init.tile([128, HC, BL], f32)
            spikeT = init.tile([128, HC, BL], f32)
            nc.vector.memset(probT, 0.0)
            nc.vector.memset(spikeT, 0.0)
            prob_state = probT
            spike_state = spikeT

            for tg in range(TG):
                xwb = blkin.tile([128, HC, TB, BL], f32, tag="xwb")
                nc.sync.dma_start(out=xwb, in_=xw_scr[tg].rearrange("h p t b -> p h t b"))
                ub = blkin.tile([128, HC, TB, BL], f32, tag="ub")
                nc.sync.dma_start(out=ub, in_=ublk[tg].rearrange("h p t b -> p h t b"))
                pstg = stg.tile([128, HC, TB, BL], f32, tag="pstg")
                sstg = stg.tile([128, HC, TB, BL], f32, tag="sstg")

                for ti in range(TB):
                    # two h-chunk groups on separate PSUM banks: group 0's
                    # elementwise chain (and its spike chunks) completes while
                    # PE still runs group 1's weight loads, so the next step's
                    # k=0..3 matmuls start without waiting for the full chain
                    pss = [psB.tile([128, GW, BL], f32, tag=f"ps{g}", name=f"ps{g}_{tg}_{ti}")
                           for g in range(NG)]
                    for g in range(NG):
                        for hg in range(GW):
                            hc = g * GW + hg
                            for k in range(KT):
                                nc.tensor.matmul(
                                    pss[g][:, hg, :],
                                    whh_sb[:, k, hc * 128:(hc + 1) * 128],
                                    spike_state[:, k, :],
                                    start=(k == 0), stop=(k == KT - 1),
                                )
                    prob_new = pstg[:, :, ti, :]
                    spike_new = sstg[:, :, ti, :]
                    for g in range(NG):
                        hs = slice(g * GW, (g + 1) * GW)
                        pre = ch.tile([128, GW, BL], f32, tag=f"pre{g}")
                        nc.vector.tensor_add(pre, xwb[:, hs, ti, :], pss[g])
                        ta = ch.tile([128, GW, BL], f32, tag=f"ta{g}")
                        nc.scalar.activation(out=ta, in_=prob_state[:, hs, :], func=ACT.Copy, scale=0.9)
                        tb_ = ch.tile([128, GW, BL], f32, tag=f"tb{g}")
                        nc.scalar.activation(out=tb_, in_=pre, func=ACT.Copy, scale=S01)
                        arg = ch.tile([128, GW, BL], f32, tag=f"arg{g}")
                        nc.vector.tensor_add(arg, ta, tb_)
                        e = ch.tile([128, GW, BL], f32, tag=f"e{g}")
                        nc.scalar.activation(out=e, in_=arg, func=ACT.Exp, scale=-1.0)
                        e2 = ch.tile([128, GW, BL], f32, tag=f"e2{g}")
                        nc.scalar.activation(out=e2, in_=e, func=ACT.Copy, scale=1.0, bias=1.0)
                        sig = ch.tile([128, GW, BL], f32, tag=f"sig{g}")
                        nc.vector.reciprocal(sig, e2)
                        nc.scalar.activation(out=prob_new[:, hs, :], in_=sig, func=ACT.Copy, scale=S099, bias=S0005)
                        samp = ch.tile([128, GW, BL], f32, tag=f"samp{g}")
                        nc.vector.tensor_tensor(samp, ub[:, hs, ti, :], prob_new[:, hs, :], ALU.is_lt)
                        s1 = ch.tile([128, GW, BL], f32, tag=f"s1{g}")
                        nc.vector.tensor_add(s1, samp, prob_new[:, hs, :])
                        nc.vector.tensor_sub(spike_new[:, hs, :], s1, prob_new[:, hs, :])
                    prob_state = prob_new
                    spike_state = spike_new

                nc.sync.dma_start(out=probs_blk[tg].rearrange("h p t b -> p h t b"), in_=pstg)
                nc.sync.dma_start(out=probs_scr[tg].rearrange("h p t b -> p h t b"), in_=pstg)
                nc.sync.dma_start(out=spikes_blk[tg].rearrange("h p t b -> p h t b"), in_=sstg)

            # ---------------- Phase C: preds ----------------
            whht_sb = wpool.tile([128, KT, H], f32, tag="w")
            nc.sync.dma_start(out=whht_sb, in_=w_hht.rearrange("(k p) h -> p k h", p=128))
            for tg in range(TG):
                pk = xkp.tile([128, KT, TB, BL], f32)
                nc.sync.dma_start(out=pk, in_=probs_scr[tg].rearrange("k p t b -> p k t b"))
                for hc in range(HC):
                    ps = psA.tile([128, TB, BL], f32)
                    for k in range(KT):
                        nc.tensor.matmul(
                            ps,
                            whht_sb[:, k, hc * 128:(hc + 1) * 128],
                            pk[:, k, :, :],
                            start=(k == 0), stop=(k == KT - 1),
                        )
                    e = cpA.tile([128, TB, BL], f32, tag="eC")
                    nc.scalar.activation(out=e, in_=ps, func=ACT.Exp, scale=-1.0)
                    e2 = cpA.tile([128, TB, BL], f32, tag="e2C")
                    nc.scalar.activation(out=e2, in_=e, func=ACT.Copy, scale=1.0, bias=1.0)
                    pr = cpA.tile([128, TB, BL], f32, tag="prC")
                    nc.vector.reciprocal(pr, e2)
                    nc.sync.dma_start(out=preds_blk[tg, hc], in_=pr)

    nc.compile()
    _BUILD_CACHE["nc"] = nc
    return nc


def _harvest_uniforms():
    """Device-RNG uniforms, bit-identical to the scan's rng_bit_generator."""
    cache_path = os.path.join(os.path.dirname(os.path.abspath(__file__)), "cache", "U_dev.npy")
    if os.path.exists(cache_path):
        return np.load(cache_path)
    import jax, jax.numpy as jnp
    keys = jax.random.split(jax.random.key(42), T)

    def uscan(keys):
        def body(c, k):
            return c, jax.random.uniform(k, (B, H), dtype=jnp.float32)
        _, us = jax.lax.scan(body, 0, keys)
        return us
    return np.asarray(jax.jit(uscan)(keys))


def _blk(a):
    """[T, H, BL] -> [TG, HC, 128, TB, BL] block layout."""
    return np.ascontiguousarray(
        a.reshape(TG, TB, HC, 128, BL).transpose(0, 2, 3, 1, 4))


def _unblk(a):
    """[TG, HC, 128, TB, BL] -> [T, BL, H]."""
    return a.transpose(0, 3, 4, 1, 2).reshape(T, BL, H)


def kernel(inputs, W_ih, W_hh):
    from concourse.bass_utils import run_bass_kernel_spmd

    inputs = np.asarray(inputs, dtype=np.float32)
    W_ih = np.ascontiguousarray(np.asarray(W_ih, dtype=np.float32))
    W_hh = np.ascontiguousarray(np.asarray(W_hh, dtype=np.float32))
    W_hhT = np.ascontiguousarray(W_hh.T)

    U = _harvest_uniforms()           # [T, B, H]
    nc = _build()

    in_maps = []
    for c in range(NC):
        bsl = slice(c * BL, (c + 1) * BL)
        xTc = np.ascontiguousarray(inputs[:, bsl, :].transpose(2, 0, 1))   # [I, T, BL]
        u_thb = U[:, bsl, :].transpose(0, 2, 1)                            # [T, H, BL]
        ublk = _blk(u_thb)
        in_maps.append({
            "xT": xTc, "ublk": ublk,
            "w_ih": W_ih, "w_hh": W_hh, "w_hht": W_hhT,
        })

    _BUILD_CACHE["last_in_maps"] = in_maps
    res = run_bass_kernel_spmd(nc, in_maps, core_ids=list(range(NC)))

    probs = np.empty((T, B, H), np.float32)
    spikes = np.empty((T + 1, B, H), np.float32)
    preds = np.empty((T, B, H), np.float32)
    spikes[0] = 0.0
    for c in range(NC):
        bsl = slice(c * BL, (c + 1) * BL)
        r = res.results[c]
        probs[:, bsl, :] = _unblk(r["probs_blk"])
        spikes[1:, bsl, :] = _unblk(r["spikes_blk"])
        preds[:, bsl, :] = _unblk(r["preds_blk"])
    prob_T = probs[-1].copy()
    spike_T = spikes[-1].copy()
    return spikes, preds, probs, prob_T, spike_T
